# revision 23
# baseline (speedup 1.0000x reference)
"""CustomLSTMCell fused kernel for 8x Trainium2 NeuronCores.

Reference computation (B=8192, D=H=1024):
    z = e_t @ W_x.T + h_prev @ W_h.T + (b_x + b_h + b_extra)   # [B, 4H]
    f, i, o, c = split(z, 4)
    c_t = sigmoid(f) * c_prev + sigmoid(i) * tanh(c)
    h_t = sigmoid(o) * tanh(c_t)

Sharding: 2-way batch x 4-way hidden-unit (8 cores, no collectives).
Each core computes z transposed ([gate_rows, batch] layout) so the bias
folds into the ScalarE activation's per-partition bias operand, and both
matmul operands arrive pre-transposed from the host (contraction dim on
partitions).

Numerics: x, W, c_prev and both outputs are bf16 (host round-trips);
accumulation is fp32 in PSUM and the gate elementwise runs fp32.
Measured rel-max-err 6.1e-3 vs the fp32 reference (tolerance 2e-2).
bf16 streams at the same 1 col/cycle as float32r but halves all DMA
traffic and gets the fast (FWL) weight-load path; the steady-state
matmul rate is ~216ns per 512-col MM (floor 512/2.4GHz + ~2.5ns NX),
with the 1024-MM stream gap-free at ~221us.

Schedule (measured ~240us total with trace; ~233.6us last-matmul):
 - One dma_start lands on ONE DMA queue (~55-100 GB/s), so first-chunk
   latency is set by per-transfer size, not aggregate bandwidth: W chunk
   0 rides the scalar ring whole (splitting it is SLOWER - the ring
   serializes its own dma_starts), chunk 1 the gpsimd ring, and the sync
   ring streams x00, x01, then w2..w15 interleaved with the rest of x so
   the PE chases the arrival stream k-outer for batch block 0.
 - 7 cold warm-up matmuls bridge PE-queue start (~8us) to chunk-0
   arrival (~11.4us); the HAM clock-gate budget is time-based, so any
   idle gap before the real stream restarts the 3.4us warm-up window.
 - Steady-state outputs ride gpsimd (SWDGE) issued right after their
   producing DVE op (an issue-blocking wait on any ring stalls that
   whole in-order queue); the last batch block's outputs ride sync,
   which has drained the input stream by then, and the very last ht is
   split across the scalar+sync rings (64 partitions each) because the
   final transfer + ~2.1us sem-reset barrier gates kernel end.
"""

import sys

if "/opt/trn_rl_repo" not in sys.path:
    sys.path.insert(0, "/opt/trn_rl_repo")

import numpy as np

import concourse.bass as bass
import concourse.mybir as mybir
from concourse import bacc
from concourse.bass_utils import run_bass_kernel_spmd
from concourse.tile import TileContext

F32 = mybir.dt.float32
F32R = mybir.dt.float32r
BF16 = mybir.dt.bfloat16
AFT = mybir.ActivationFunctionType
ALU = mybir.AluOpType

B, D, H = 8192, 1024, 1024
M_BATCH, M_UNIT = 2, 4          # batch split x unit split = 8 cores
BS = B // M_BATCH               # 4096 batch rows per core
U = H // M_UNIT                 # 256 hidden units per core
K = D + H                       # 2048 contraction (e_t | h_prev)
KT = K // 128                   # 16 k-chunks
G = 4 * U                       # 1024 gate rows per core (f|i|o|c x U)
BBLK = 512                      # moving free-dim per matmul
NBB = BS // BBLK                # 8 batch blocks
NJ = U // 128                   # 2 unit sub-blocks of 128 partitions

GATE_FUNCS = [AFT.Sigmoid, AFT.Sigmoid, AFT.Sigmoid, AFT.Tanh]  # f, i, o, c


def _build_nc():
    nc = bacc.Bacc()

    xT = nc.dram_tensor("xT", [K, BS], BF16, kind="ExternalInput")
    wT = nc.dram_tensor("wT", [K, G], BF16, kind="ExternalInput")
    bias = nc.dram_tensor("bias", [G], F32, kind="ExternalInput")
    cT = nc.dram_tensor("cT", [U, BS], BF16, kind="ExternalInput")
    hT_out = nc.dram_tensor("hT_out", [U, BS], BF16, kind="ExternalOutput")
    cT_out = nc.dram_tensor("cT_out", [U, BS], BF16, kind="ExternalOutput")

    xT_r = xT.ap().rearrange("(k p) b -> p k b", p=128)      # [128, KT, BS]
    wT_r = wT.ap().rearrange("(k p) g -> p k g", p=128)      # [128, KT, G]
    bias_r = bias.ap().rearrange("(c p) -> p c", p=128)      # [128, 4*NJ]
    cT_r = cT.ap().rearrange("(j p) b -> p j b", p=128)      # [128, NJ, BS]
    hT_r = hT_out.ap().rearrange("(j p) b -> p j b", p=128)
    cTo_r = cT_out.ap().rearrange("(j p) b -> p j b", p=128)

    with TileContext(nc) as tc:
        with (
            tc.tile_pool(name="wpool", bufs=1) as wpool,
            tc.tile_pool(name="xpool", bufs=2) as xpool,
            tc.tile_pool(name="cpool", bufs=2) as cpool,
            tc.tile_pool(name="gpool", bufs=2) as gpool,
            tc.tile_pool(name="opool", bufs=2) as opool,
            tc.tile_pool(name="psum", bufs=2, space="PSUM") as pp,
        ):
            # Per-k W tiles, DMA-interleaved with the first batch block's x
            # tiles so the PE can start on chunk 0 immediately and chase the
            # arrival stream.  All on the sync ring: its ~650ns/DMA issue
            # rate paces arrivals in k-order, and the scalar engine must stay
            # free for ACTIVATEs (a DMA chain there develops issue-blocking
            # sem waits that stall PSUM release by ~8us).
            # Chunk 0 rides the scalar ring and chunk 1 the gpsimd ring so
            # they land in parallel with the sync ring's x00/x01 (measured:
            # chunk 0 complete ~10.5us, real matmuls from ~11us).
            wt0 = wpool.tile([128, G], BF16, tag="w0", name="w0")
            wt1 = wpool.tile([128, G], BF16, tag="w1", name="w1")
            nc.scalar.dma_start(out=wt0[:], in_=wT_r[:, 0, :])
            bias_sb = wpool.tile([128, 4 * NJ], F32)
            nc.scalar.dma_start(out=bias_sb[:], in_=bias_r)

            # PE warm-up: ~12 throwaway matmuls on zeroed scratch while the
            # first W/x chunks are still in flight.  The HAM clock gate needs
            # ~3.4us of sustained PE activity to lift the PE from 1.2GHz to
            # 2.4GHz; without this the first ~13 real matmuls run cold.
            # Half-width warm-up tile: its memset is ~3x cheaper than a
            # full-width one, so the PE starts earlier and the HAM 3.4us
            # busy-window completes as chunk 0 lands -- the real stream
            # then starts at (or near) the full 2.4GHz clock.
            warm = wpool.tile([128, 256], BF16, name="warm")
            nc.vector.memset(warm[:], 0.0)
            warm_ps = pp.tile([128, BBLK], F32, tag="ps0", name="warm_ps")
            # 14 cold N=256 matmuls (~213ns each) bridge PE-queue start
            # (~7.5us) to chunk-0 arrival (~10.5-11us).
            for _ in range(14):
                nc.tensor.matmul(
                    warm_ps[:, 0:256], warm[:, 0:128], warm[:], start=True, stop=True
                )

            w_sb = []
            x0_sb = []
            w_sb += [wt0, wt1]
            nc.sync.dma_start(out=wt1[:], in_=wT_r[:, 1, :])
            for k in range(KT):
                if k >= 2:
                    wt = wpool.tile([128, G], BF16, tag=f"w{k}", name=f"w{k}")
                    nc.sync.dma_start(out=wt[:], in_=wT_r[:, k, :])
                    w_sb.append(wt)
                if k % 2 == 0:
                    xp = xpool.tile(
                        [128, 2, BBLK], BF16, tag=f"xp{k // 2}", name=f"xp{k // 2}", bufs=3
                    )
                nc.sync.dma_start(out=xp[:, k % 2, :], in_=xT_r[:, k, 0:BBLK])
                x0_sb.append(xp[:, k % 2, :])

            def load_cprev(bb):
                t = cpool.tile([128, NJ, BBLK], BF16, tag="cprev", name="cprev")
                nc.scalar.dma_start(
                    out=t[:], in_=cT_r[:, :, bb * BBLK:(bb + 1) * BBLK]
                )
                return t

            def elementwise(ps, cprev_sb, bb, j, n_split=1, out_ring=None,
                            final=False, base=0, width=BBLK):
                """Gate nonlinearities + cell update for one quadruple.

                c_t's DMA issues as soon as c_t is computed (it does not
                depend on the o-gate), shortening the output tail.  The last
                batch block's outputs ride the sync ring, which has finished
                the x input stream by then -- the gpsimd ring otherwise
                accumulates a ~16us blocked-issue backlog that gates the
                kernel end.
                """
                out_ring = out_ring or nc.scalar
                w = width // n_split
                for s in range(n_split):
                    csl = slice(s * w, (s + 1) * w)

                    def gate(g):
                        at = gpool.tile([128, w], F32, tag=f"act{g}", name=f"act{g}")
                        nc.scalar.activation(
                            at[:], ps[g][:, csl], GATE_FUNCS[g],
                            bias=bias_sb[:, 2 * g + j: 2 * g + j + 1],
                        )
                        return at

                    # ACT stream order mirrors the (c,f,i,o) matmul order so
                    # the c_t chain completes before the o-gate's matmuls do.
                    mc = gate(3)
                    gf = gate(0)
                    gi = gate(1)
                    t1 = gpool.tile([128, w], F32, tag="t1", name="t1")
                    nc.vector.tensor_tensor(
                        t1[:], gf[:], cprev_sb[:, j, base + s * w: base + (s + 1) * w],
                        ALU.mult,
                    )
                    t2 = gpool.tile([128, w], F32, tag="t2", name="t2")
                    nc.vector.tensor_tensor(t2[:], gi[:], mc[:], ALU.mult)
                    ct = opool.tile([128, w], BF16, tag="ct", name="ct")
                    nc.vector.tensor_tensor(ct[:], t1[:], t2[:], ALU.add)
                    osl = slice(bb * BBLK + base + s * w,
                                bb * BBLK + base + (s + 1) * w)
                    out_ring.dma_start(out=cTo_r[:, j, osl], in_=ct[:])
                    th = gpool.tile([128, w], F32, tag="th", name="th")
                    nc.scalar.activation(th[:], ct[:], AFT.Tanh)
                    go = gate(2)
                    ht = opool.tile([128, w], BF16, tag="ht", name="ht")
                    nc.vector.tensor_tensor(ht[:], go[:], th[:], ALU.mult)
                    if final and s == n_split - 1:
                        # The very last transfer gates the epilogue barrier:
                        # split the final ht across the two HWDGE rings.
                        nc.scalar.dma_start(out=hT_r[0:64, j, osl], in_=ht[0:64, :])
                        nc.sync.dma_start(out=hT_r[64:128, j, osl], in_=ht[64:128, :])
                    else:
                        out_ring.dma_start(out=hT_r[:, j, osl], in_=ht[:])

            # ---- batch block 0: k-outer over both j's, chasing the DMA
            # stream (8 MMs per arriving k-chunk matches the ~2.2us/chunk
            # delivery rate) ----
            cprev0 = load_cprev(0)
            ps0 = [
                [pp.tile([128, BBLK], F32, tag=f"ps{g}", name=f"ps{g}") for g in range(4)]
                for j in range(NJ)
            ]
            for k in range(KT):
                for j in range(NJ):
                    for g in (3, 0, 1, 2):  # c-gate first: longest elementwise chain
                        nc.tensor.matmul(
                            ps0[j][g][:],
                            w_sb[k][:, g * U + j * 128: g * U + (j + 1) * 128],
                            x0_sb[k][:],
                            start=(k == 0),
                            stop=(k == KT - 1),
                        )
            for j in range(NJ):
                elementwise(ps0[j], cprev0, 0, j)

            # ---- batch blocks 1..NBB-1: gate-outer, k-inner ----
            for bb in range(1, NBB):
                bsl = slice(bb * BBLK, (bb + 1) * BBLK)
                x_sb = []
                for kp in range(KT // 2):
                    xt = xpool.tile(
                        [128, 2, BBLK], BF16, tag=f"xp{kp}", name=f"xp{kp}", bufs=3
                    )
                    nc.sync.dma_start(out=xt[:], in_=xT_r[:, 2 * kp:2 * kp + 2, bsl])
                    x_sb.extend([xt[:, 0, :], xt[:, 1, :]])
                cprev_sb = load_cprev(bb)

                for j in range(NJ):
                    last_j = (bb == NBB - 1) and (j == NJ - 1)
                    if not last_j:
                        ps = [None] * 4
                        for g in (3, 0, 1, 2):  # c-gate first: longest chain
                            pst = pp.tile([128, BBLK], F32, tag=f"ps{g}", name=f"ps{g}")
                            col0 = g * U + j * 128
                            for k in range(KT):
                                nc.tensor.matmul(
                                    pst[:],
                                    w_sb[k][:, col0:col0 + 128],
                                    x_sb[k][:],
                                    start=(k == 0),
                                    stop=(k == KT - 1),
                                )
                            ps[g] = pst
                        elementwise(
                            ps, cprev_sb, bb, j,
                            out_ring=nc.sync if bb == NBB - 1 else None,
                        )
                    else:
                        # Final quadruple runs as two 256-col half-quadruples
                        # so the post-matmul ACT/DVE chain and the very last
                        # transfer (which gate the epilogue barrier) are half
                        # as long.
                        HB = BBLK // 2
                        for hb in range(2):
                            bsl2 = slice(hb * HB, (hb + 1) * HB)
                            ps = [None] * 4
                            for g in (3, 0, 1, 2):
                                # full-width PSUM tile (pool slot reuse);
                                # only the first 256 cols are written/read
                                pst = pp.tile(
                                    [128, BBLK], F32, tag=f"ps{g}", name=f"ps{g}"
                                )
                                col0 = g * U + j * 128
                                for k in range(KT):
                                    nc.tensor.matmul(
                                        pst[:, 0:HB],
                                        w_sb[k][:, col0:col0 + 128],
                                        x_sb[k][:, bsl2],
                                        start=(k == 0),
                                        stop=(k == KT - 1),
                                    )
                                ps[g] = pst
                            elementwise(
                                ps, cprev_sb, bb, j,
                                out_ring=nc.sync,
                                final=(hb == 1),
                                base=hb * HB, width=HB,
                            )

    nc.finalize()
    return nc


def _shard_inputs(e_t, h_prev, c_prev, W_x, b_x, W_h, b_h, b_extra):
    import ml_dtypes
    BF = ml_dtypes.bfloat16
    e_t = np.ascontiguousarray(np.asarray(e_t, dtype=np.float32))
    h_prev = np.ascontiguousarray(np.asarray(h_prev, dtype=np.float32))
    c_prev = np.ascontiguousarray(np.asarray(c_prev, dtype=np.float32))
    W_x = np.asarray(W_x, dtype=np.float32)
    W_h = np.asarray(W_h, dtype=np.float32)
    bias_full = (
        np.asarray(b_x, dtype=np.float32)
        + np.asarray(b_h, dtype=np.float32)
        + np.asarray(b_extra, dtype=np.float32)
    )

    # X^T = [e_t | h_prev]^T : [K, B], cast to bf16 for the PE fast path
    XT = np.empty((K, B), dtype=BF)
    XT[:D] = e_t.T.astype(BF)
    XT[D:] = h_prev.T.astype(BF)
    W = np.concatenate([W_x, W_h], axis=1).astype(BF)  # [4H, K]

    in_maps = []
    for core in range(M_BATCH * M_UNIT):
        m, q = divmod(core, M_UNIT)
        rows = np.concatenate(
            [np.arange(g0 + q * U, g0 + (q + 1) * U) for g0 in (0, H, 2 * H, 3 * H)]
        )
        in_maps.append({
            "xT": np.ascontiguousarray(XT[:, m * BS:(m + 1) * BS]),
            "wT": np.ascontiguousarray(W[rows].T),
            "bias": np.ascontiguousarray(bias_full[rows]),
            "cT": np.ascontiguousarray(
                c_prev[m * BS:(m + 1) * BS, q * U:(q + 1) * U].T.astype(BF)
            ),
        })
    return in_maps


def _assemble_outputs(results):
    h_t = np.empty((B, H), dtype=np.float32)
    c_t = np.empty((B, H), dtype=np.float32)
    for core, res in enumerate(results):
        m, q = divmod(core, M_UNIT)
        h_t[m * BS:(m + 1) * BS, q * U:(q + 1) * U] = res["hT_out"].T.astype(np.float32)
        c_t[m * BS:(m + 1) * BS, q * U:(q + 1) * U] = res["cT_out"].T.astype(np.float32)
    return h_t, c_t


def kernel(e_t, h_prev, c_prev, W_x, b_x, W_h, b_h, b_extra, _runner=None):
    in_maps = _shard_inputs(e_t, h_prev, c_prev, W_x, b_x, W_h, b_h, b_extra)
    nc = _build_nc()
    if _runner is None:
        res = run_bass_kernel_spmd(nc, in_maps, core_ids=list(range(8)))
        results = res.results
    else:
        results = _runner(nc, in_maps)
    return _assemble_outputs(results)



# revision 25
# speedup vs baseline: 1.0028x; 1.0028x over previous
"""CustomLSTMCell fused kernel for 8x Trainium2 NeuronCores.

Reference computation (B=8192, D=H=1024):
    z = e_t @ W_x.T + h_prev @ W_h.T + (b_x + b_h + b_extra)   # [B, 4H]
    f, i, o, c = split(z, 4)
    c_t = sigmoid(f) * c_prev + sigmoid(i) * tanh(c)
    h_t = sigmoid(o) * tanh(c_t)

Sharding: 2-way batch x 4-way hidden-unit (8 cores, no collectives).
Each core computes z transposed ([gate_rows, batch] layout) so the bias
folds into the ScalarE activation's per-partition bias operand, and both
matmul operands arrive pre-transposed from the host (contraction dim on
partitions).

Numerics: x, W, c_prev and both outputs are bf16 (host round-trips);
accumulation is fp32 in PSUM and the gate elementwise runs fp32.
Measured rel-max-err 6.1e-3 vs the fp32 reference (tolerance 2e-2).
bf16 streams at the same 1 col/cycle as float32r but halves all DMA
traffic and gets the fast (FWL) weight-load path; the steady-state
matmul rate is ~216ns per 512-col MM (floor 512/2.4GHz + ~2.5ns NX),
with the 1024-MM stream gap-free at ~221us.

Schedule (measured ~240us total with trace; ~233.6us last-matmul):
 - One dma_start lands on ONE DMA queue (~55-100 GB/s), so first-chunk
   latency is set by per-transfer size, not aggregate bandwidth: W chunk
   0 rides the scalar ring whole (splitting it is SLOWER - the ring
   serializes its own dma_starts), chunk 1 the gpsimd ring, and the sync
   ring streams x00, x01, then w2..w15 interleaved with the rest of x so
   the PE chases the arrival stream k-outer for batch block 0.
 - 7 cold warm-up matmuls bridge PE-queue start (~8us) to chunk-0
   arrival (~11.4us); the HAM clock-gate budget is time-based, so any
   idle gap before the real stream restarts the 3.4us warm-up window.
 - Steady-state outputs ride gpsimd (SWDGE) issued right after their
   producing DVE op (an issue-blocking wait on any ring stalls that
   whole in-order queue); the last batch block's outputs ride sync,
   which has drained the input stream by then, and the very last ht is
   split across the scalar+sync rings (64 partitions each) because the
   final transfer + ~2.1us sem-reset barrier gates kernel end.
"""

import sys

if "/opt/trn_rl_repo" not in sys.path:
    sys.path.insert(0, "/opt/trn_rl_repo")

import numpy as np

import concourse.bass as bass
import concourse.mybir as mybir
from concourse import bacc
from concourse.bass_utils import run_bass_kernel_spmd
from concourse.tile import TileContext

F32 = mybir.dt.float32
F32R = mybir.dt.float32r
BF16 = mybir.dt.bfloat16
AFT = mybir.ActivationFunctionType
ALU = mybir.AluOpType

B, D, H = 8192, 1024, 1024
M_BATCH, M_UNIT = 2, 4          # batch split x unit split = 8 cores
BS = B // M_BATCH               # 4096 batch rows per core
U = H // M_UNIT                 # 256 hidden units per core
K = D + H                       # 2048 contraction (e_t | h_prev)
KT = K // 128                   # 16 k-chunks
G = 4 * U                       # 1024 gate rows per core (f|i|o|c x U)
BBLK = 512                      # moving free-dim per matmul
NBB = BS // BBLK                # 8 batch blocks
NJ = U // 128                   # 2 unit sub-blocks of 128 partitions

GATE_FUNCS = [AFT.Sigmoid, AFT.Sigmoid, AFT.Sigmoid, AFT.Tanh]  # f, i, o, c


def _build_nc():
    nc = bacc.Bacc()

    xT = nc.dram_tensor("xT", [K, BS], BF16, kind="ExternalInput")
    wT = nc.dram_tensor("wT", [K, G], BF16, kind="ExternalInput")
    bias = nc.dram_tensor("bias", [G], F32, kind="ExternalInput")
    cT = nc.dram_tensor("cT", [U, BS], BF16, kind="ExternalInput")
    hT_out = nc.dram_tensor("hT_out", [U, BS], BF16, kind="ExternalOutput")
    cT_out = nc.dram_tensor("cT_out", [U, BS], BF16, kind="ExternalOutput")

    xT_r = xT.ap().rearrange("(k p) b -> p k b", p=128)      # [128, KT, BS]
    wT_r = wT.ap().rearrange("(k p) g -> p k g", p=128)      # [128, KT, G]
    bias_r = bias.ap().rearrange("(c p) -> p c", p=128)      # [128, 4*NJ]
    cT_r = cT.ap().rearrange("(j p) b -> p j b", p=128)      # [128, NJ, BS]
    hT_r = hT_out.ap().rearrange("(j p) b -> p j b", p=128)
    cTo_r = cT_out.ap().rearrange("(j p) b -> p j b", p=128)

    with TileContext(nc) as tc:
        with (
            tc.tile_pool(name="wpool", bufs=1) as wpool,
            tc.tile_pool(name="xpool", bufs=2) as xpool,
            tc.tile_pool(name="cpool", bufs=2) as cpool,
            tc.tile_pool(name="gpool", bufs=2) as gpool,
            tc.tile_pool(name="opool", bufs=2) as opool,
            tc.tile_pool(name="psum", bufs=2, space="PSUM") as pp,
        ):
            # Per-k W tiles, DMA-interleaved with the first batch block's x
            # tiles so the PE can start on chunk 0 immediately and chase the
            # arrival stream.  All on the sync ring: its ~650ns/DMA issue
            # rate paces arrivals in k-order, and the scalar engine must stay
            # free for ACTIVATEs (a DMA chain there develops issue-blocking
            # sem waits that stall PSUM release by ~8us).
            # Chunk 0 rides the scalar ring and chunk 1 the gpsimd ring so
            # they land in parallel with the sync ring's x00/x01 (measured:
            # chunk 0 complete ~10.5us, real matmuls from ~11us).
            # w0 heads the sync ring (fastest queue startup: a first-issued
            # 128KB transfer completes ~8.9us there); w1 + bias ride scalar.
            wt0 = wpool.tile([128, G], BF16, tag="w0", name="w0")
            wt1 = wpool.tile([128, G], BF16, tag="w1", name="w1")
            nc.sync.dma_start(out=wt0[:], in_=wT_r[:, 0, :])
            nc.scalar.dma_start(out=wt1[:], in_=wT_r[:, 1, :])
            bias_sb = wpool.tile([128, 4 * NJ], F32)
            nc.scalar.dma_start(out=bias_sb[:], in_=bias_r)

            # PE warm-up: ~12 throwaway matmuls on zeroed scratch while the
            # first W/x chunks are still in flight.  The HAM clock gate needs
            # ~3.4us of sustained PE activity to lift the PE from 1.2GHz to
            # 2.4GHz; without this the first ~13 real matmuls run cold.
            warm = wpool.tile([128, BBLK], BF16, name="warm")
            nc.vector.memset(warm[:], 0.0)
            warm_r = warm
            warm_ps = pp.tile([128, BBLK], F32, tag="ps0", name="warm_ps")
            # A few cold-rate matmuls bridge the gap until the first W/x
            # chunks land; the HAM warm-up budget is time-based, so any
            # further warm-up work would only delay the real stream.
            for _ in range(6):
                nc.tensor.matmul(
                    warm_ps[:], warm_r[:, 0:128], warm_r[:], start=True, stop=True
                )

            w_sb = []
            x0_sb = []
            w_sb += [wt0, wt1]
            for k in range(KT):
                if k >= 2:
                    wt = wpool.tile([128, G], BF16, tag=f"w{k}", name=f"w{k}")
                    nc.sync.dma_start(out=wt[:], in_=wT_r[:, k, :])
                    w_sb.append(wt)
                if k % 2 == 0:
                    xp = xpool.tile(
                        [128, 2, BBLK], BF16, tag=f"xp{k // 2}", name=f"xp{k // 2}", bufs=3
                    )
                nc.sync.dma_start(out=xp[:, k % 2, :], in_=xT_r[:, k, 0:BBLK])
                x0_sb.append(xp[:, k % 2, :])

            def load_cprev(bb):
                t = cpool.tile([128, NJ, BBLK], BF16, tag="cprev", name="cprev")
                nc.scalar.dma_start(
                    out=t[:], in_=cT_r[:, :, bb * BBLK:(bb + 1) * BBLK]
                )
                return t

            def elementwise(ps, cprev_sb, bb, j, n_split=1, out_ring=None,
                            final=False, base=0, width=BBLK):
                """Gate nonlinearities + cell update for one quadruple.

                c_t's DMA issues as soon as c_t is computed (it does not
                depend on the o-gate), shortening the output tail.  The last
                batch block's outputs ride the sync ring, which has finished
                the x input stream by then -- the gpsimd ring otherwise
                accumulates a ~16us blocked-issue backlog that gates the
                kernel end.
                """
                out_ring = out_ring or nc.scalar
                w = width // n_split
                for s in range(n_split):
                    csl = slice(s * w, (s + 1) * w)

                    def gate(g):
                        at = gpool.tile([128, w], F32, tag=f"act{g}", name=f"act{g}")
                        nc.scalar.activation(
                            at[:], ps[g][:, csl], GATE_FUNCS[g],
                            bias=bias_sb[:, 2 * g + j: 2 * g + j + 1],
                        )
                        return at

                    # ACT stream order mirrors the (c,f,i,o) matmul order so
                    # the c_t chain completes before the o-gate's matmuls do.
                    mc = gate(3)
                    gf = gate(0)
                    gi = gate(1)
                    t1 = gpool.tile([128, w], F32, tag="t1", name="t1")
                    nc.vector.tensor_tensor(
                        t1[:], gf[:], cprev_sb[:, j, base + s * w: base + (s + 1) * w],
                        ALU.mult,
                    )
                    t2 = gpool.tile([128, w], F32, tag="t2", name="t2")
                    nc.vector.tensor_tensor(t2[:], gi[:], mc[:], ALU.mult)
                    ct = opool.tile([128, w], BF16, tag="ct", name="ct")
                    nc.vector.tensor_tensor(ct[:], t1[:], t2[:], ALU.add)
                    osl = slice(bb * BBLK + base + s * w,
                                bb * BBLK + base + (s + 1) * w)
                    out_ring.dma_start(out=cTo_r[:, j, osl], in_=ct[:])
                    th = gpool.tile([128, w], F32, tag="th", name="th")
                    nc.scalar.activation(th[:], ct[:], AFT.Tanh)
                    go = gate(2)
                    ht = opool.tile([128, w], BF16, tag="ht", name="ht")
                    nc.vector.tensor_tensor(ht[:], go[:], th[:], ALU.mult)
                    if final and s == n_split - 1:
                        # The very last transfer gates the epilogue barrier:
                        # split the final ht across the two HWDGE rings.
                        nc.scalar.dma_start(out=hT_r[0:64, j, osl], in_=ht[0:64, :])
                        nc.sync.dma_start(out=hT_r[64:128, j, osl], in_=ht[64:128, :])
                    else:
                        out_ring.dma_start(out=hT_r[:, j, osl], in_=ht[:])

            # ---- batch block 0: k-outer over both j's, chasing the DMA
            # stream (8 MMs per arriving k-chunk matches the ~2.2us/chunk
            # delivery rate) ----
            cprev0 = load_cprev(0)
            ps0 = [
                [pp.tile([128, BBLK], F32, tag=f"ps{g}", name=f"ps{g}") for g in range(4)]
                for j in range(NJ)
            ]
            for k in range(KT):
                for j in range(NJ):
                    for g in (3, 0, 1, 2):  # c-gate first: longest elementwise chain
                        nc.tensor.matmul(
                            ps0[j][g][:],
                            w_sb[k][:, g * U + j * 128: g * U + (j + 1) * 128],
                            x0_sb[k][:],
                            start=(k == 0),
                            stop=(k == KT - 1),
                        )
            for j in range(NJ):
                elementwise(ps0[j], cprev0, 0, j)

            # ---- batch blocks 1..NBB-1: gate-outer, k-inner ----
            for bb in range(1, NBB):
                bsl = slice(bb * BBLK, (bb + 1) * BBLK)
                x_sb = []
                for kp in range(KT // 2):
                    xt = xpool.tile(
                        [128, 2, BBLK], BF16, tag=f"xp{kp}", name=f"xp{kp}", bufs=3
                    )
                    nc.sync.dma_start(out=xt[:], in_=xT_r[:, 2 * kp:2 * kp + 2, bsl])
                    x_sb.extend([xt[:, 0, :], xt[:, 1, :]])
                cprev_sb = load_cprev(bb)

                for j in range(NJ):
                    last_j = (bb == NBB - 1) and (j == NJ - 1)
                    if not last_j:
                        ps = [None] * 4
                        for g in (3, 0, 1, 2):  # c-gate first: longest chain
                            pst = pp.tile([128, BBLK], F32, tag=f"ps{g}", name=f"ps{g}")
                            col0 = g * U + j * 128
                            for k in range(KT):
                                nc.tensor.matmul(
                                    pst[:],
                                    w_sb[k][:, col0:col0 + 128],
                                    x_sb[k][:],
                                    start=(k == 0),
                                    stop=(k == KT - 1),
                                )
                            ps[g] = pst
                        elementwise(
                            ps, cprev_sb, bb, j,
                            out_ring=nc.sync if bb == NBB - 1 else None,
                        )
                    else:
                        # Final quadruple runs as two 256-col half-quadruples
                        # so the post-matmul ACT/DVE chain and the very last
                        # transfer (which gate the epilogue barrier) are half
                        # as long.
                        HB = BBLK // 2
                        for hb in range(2):
                            bsl2 = slice(hb * HB, (hb + 1) * HB)
                            ps = [None] * 4
                            for g in (3, 0, 1, 2):
                                # full-width PSUM tile (pool slot reuse);
                                # only the first 256 cols are written/read
                                pst = pp.tile(
                                    [128, BBLK], F32, tag=f"ps{g}", name=f"ps{g}"
                                )
                                col0 = g * U + j * 128
                                for k in range(KT):
                                    nc.tensor.matmul(
                                        pst[:, 0:HB],
                                        w_sb[k][:, col0:col0 + 128],
                                        x_sb[k][:, bsl2],
                                        start=(k == 0),
                                        stop=(k == KT - 1),
                                    )
                                ps[g] = pst
                            elementwise(
                                ps, cprev_sb, bb, j,
                                out_ring=nc.sync,
                                final=(hb == 1),
                                base=hb * HB, width=HB,
                            )

    nc.finalize()
    return nc


def _shard_inputs(e_t, h_prev, c_prev, W_x, b_x, W_h, b_h, b_extra):
    import ml_dtypes
    BF = ml_dtypes.bfloat16
    e_t = np.ascontiguousarray(np.asarray(e_t, dtype=np.float32))
    h_prev = np.ascontiguousarray(np.asarray(h_prev, dtype=np.float32))
    c_prev = np.ascontiguousarray(np.asarray(c_prev, dtype=np.float32))
    W_x = np.asarray(W_x, dtype=np.float32)
    W_h = np.asarray(W_h, dtype=np.float32)
    bias_full = (
        np.asarray(b_x, dtype=np.float32)
        + np.asarray(b_h, dtype=np.float32)
        + np.asarray(b_extra, dtype=np.float32)
    )

    # X^T = [e_t | h_prev]^T : [K, B], cast to bf16 for the PE fast path
    XT = np.empty((K, B), dtype=BF)
    XT[:D] = e_t.T.astype(BF)
    XT[D:] = h_prev.T.astype(BF)
    W = np.concatenate([W_x, W_h], axis=1).astype(BF)  # [4H, K]

    in_maps = []
    for core in range(M_BATCH * M_UNIT):
        m, q = divmod(core, M_UNIT)
        rows = np.concatenate(
            [np.arange(g0 + q * U, g0 + (q + 1) * U) for g0 in (0, H, 2 * H, 3 * H)]
        )
        in_maps.append({
            "xT": np.ascontiguousarray(XT[:, m * BS:(m + 1) * BS]),
            "wT": np.ascontiguousarray(W[rows].T),
            "bias": np.ascontiguousarray(bias_full[rows]),
            "cT": np.ascontiguousarray(
                c_prev[m * BS:(m + 1) * BS, q * U:(q + 1) * U].T.astype(BF)
            ),
        })
    return in_maps


def _assemble_outputs(results):
    h_t = np.empty((B, H), dtype=np.float32)
    c_t = np.empty((B, H), dtype=np.float32)
    for core, res in enumerate(results):
        m, q = divmod(core, M_UNIT)
        h_t[m * BS:(m + 1) * BS, q * U:(q + 1) * U] = res["hT_out"].T.astype(np.float32)
        c_t[m * BS:(m + 1) * BS, q * U:(q + 1) * U] = res["cT_out"].T.astype(np.float32)
    return h_t, c_t


def kernel(e_t, h_prev, c_prev, W_x, b_x, W_h, b_h, b_extra, _runner=None):
    in_maps = _shard_inputs(e_t, h_prev, c_prev, W_x, b_x, W_h, b_h, b_extra)
    nc = _build_nc()
    if _runner is None:
        res = run_bass_kernel_spmd(nc, in_maps, core_ids=list(range(8)))
        results = res.results
    else:
        results = _runner(nc, in_maps)
    return _assemble_outputs(results)

